# revision 2
# baseline (speedup 1.0000x reference)
"""GCN (3-layer) + mean-pool + MLP head on 8 Trainium2 NeuronCores.

Strategy (data-parallel over dst nodes):
- Nodes are partitioned into 8 contiguous ranges (one per core). Each core owns
  all edges whose dst falls in its range (plus self-loops), so the scatter side
  of message passing is core-local.
- Per layer: each core computes h = x @ W for its own nodes (bf16), the shards
  are AllGather'd into full node-major tables in DRAM, and per-edge messages
  h[src] * norm are fetched with dma_gather (SWDGE indexed DMA, 256B rows).
- Segment-sum by dst runs on the tensor engine: for each 128-edge chunk a
  selection matrix Sel[e, slot] = norm[e] * (slot[e] == s) is built with one
  vector-engine tensor_scalar op, then matmul(Sel^T @ msgs) accumulates into a
  PSUM tile per 128-node dst tile; per-quarter partials are drained into an
  SBUF accumulator (ACT copy for the first quarter, DVE adds after).
- The node table is split into FOUR tile-aligned quarters, each AllGather'd
  separately, and edges are swept quarter-major: quarter q's edge processing
  overlaps the AllGather of quarter q+1, and the h-matmul + first AllGather of
  the next layer overlaps the tail of the previous edge sweep.
- Layer 1 skips the h table: A@pos is aggregated first (messages are only
  D=3 floats, gathered from a 256B-padded f32 pos table supplied as input),
  then pushed through W1.
- Graph mean-pool: per-core partial sums via matmul against a host-built
  node->graph selection, AllReduce, then the tiny MLP head runs redundantly
  on every core.
- The SAME NEFF runs on all 8 cores (SPMD): per-(tile, quarter) edge counts
  are padded to the max across cores so the instruction schedule is identical
  everywhere; padded slots carry norm=0 so they contribute nothing.
- int16 gather indices only reach 32767, so each quarter table keeps its own
  index base (quarter size * 8 cores <= 32768 rows).
"""

import os
import numpy as np
import ml_dtypes

CORES = 8
NGRP = 4      # table quarters
MAXNI = 1024  # max indices per dma_gather call (descriptor-ring limit)
POSW = 64     # padded f32 columns of the pos gather table (256B rows)


# ----------------------------------------------------------------- host prep

def _plan_calls(m):
    """Split m (multiple of 128) indices into balanced calls of <= MAXNI."""
    if m == 0:
        return []
    q = m // 128
    ncalls = (m + MAXNI - 1) // MAXNI
    sizes = []
    base, rem = divmod(q, ncalls)
    for i in range(ncalls):
        sizes.append((base + (1 if i < rem else 0)) * 128)
    return [s for s in sizes if s > 0]


def _preprocess(pos, edge_index, batch, num_graphs):
    N, D = pos.shape
    G = int(num_graphs)
    assert N % CORES == 0, N
    npc = N // CORES                       # real nodes per core
    TR = (npc + 127) // 128                # real dst tiles per core
    TQ = (TR + NGRP - 1) // NGRP           # tiles per quarter
    T = TQ * NGRP                          # padded tile count
    npc_pad = T * 128
    qsize = TQ * 128                       # rows per core per quarter
    tq = qsize * CORES                     # rows per quarter table
    assert tq <= 32768

    src = np.concatenate([edge_index[0], np.arange(N, dtype=np.int64)])
    dst = np.concatenate([edge_index[1], np.arange(N, dtype=np.int64)])
    deg = np.bincount(dst, minlength=N).astype(np.float32)
    dinv = (1.0 / np.sqrt(np.maximum(deg, 1e-12))).astype(np.float32)
    norm = dinv[src] * dinv[dst]

    core_of = dst // npc
    loc = dst - core_of * npc
    tile_of = loc // 128
    slot_of = loc - tile_of * 128

    # remapped quarter-table row of each node
    c_n = np.arange(N, dtype=np.int64) // npc
    i_n = np.arange(N, dtype=np.int64) - c_n * npc
    q_n = i_n // qsize
    row_n = c_n * qsize + (i_n - q_n * qsize)   # row within quarter table
    grp = q_n[src]
    idx16 = row_n[src]

    # bucket edges by (group, tile, core) -- group-major device sweep order
    order = np.lexsort((idx16, slot_of, core_of, tile_of, grp))
    core_s = core_of[order]
    tile_s = tile_of[order]
    grp_s = grp[order]
    idx_s = idx16[order]
    slot_s = slot_of[order]
    norm_s = norm[order]

    key = (grp_s * TR + tile_s) * CORES + core_s
    nk = NGRP * TR * CORES
    cnt = np.bincount(key, minlength=nk).reshape(NGRP, TR, CORES)
    M = cnt.max(axis=2)                    # [NGRP, TR]
    M = ((M + 127) // 128) * 128

    # call plan shared by all cores / layers, group-major
    plan = []                              # (g, t, [ni...])
    for g in range(NGRP):
        for t in range(TR):
            plan.append((g, t, _plan_calls(int(M[g, t]))))
    tot = int(M.sum())
    nchunk = tot // 128

    starts = np.zeros(nk + 1, dtype=np.int64)
    np.cumsum(np.bincount(key, minlength=nk), out=starts[1:])
    idx_in = np.zeros((CORES, tot), dtype=np.int64)
    slot_in = np.zeros((CORES, tot), dtype=np.float32)
    norm_in = np.zeros((CORES, tot), dtype=np.float32)
    for c in range(CORES):
        off = 0
        for g in range(NGRP):
            for t in range(TR):
                k = (g * TR + t) * CORES + c
                s, e = starts[k], starts[k + 1]
                m = int(M[g, t])
                idx_in[c, off:off + (e - s)] = idx_s[s:e]
                slot_in[c, off:off + (e - s)] = slot_s[s:e]
                norm_in[c, off:off + (e - s)] = norm_s[s:e]
                off += m
    # wrap indices per call: position i of a call reads tile16[i%16, i//16]
    idx_wrapped = np.zeros((CORES, 16, tot // 16), dtype=np.int16)
    col = 0
    off = 0
    for (g, t, sizes) in plan:
        for ni in sizes:
            seg = idx_in[:, off:off + ni]
            w = seg.reshape(CORES, ni // 16, 16).transpose(0, 2, 1)
            idx_wrapped[:, :, col:col + ni // 16] = w.astype(np.int16)
            col += ni // 16
            off += ni
    idxs_inp = np.tile(idx_wrapped, (1, 8, 1))

    ns_inp = np.zeros((CORES, 128, max(nchunk, 1) * 2), dtype=np.float32)
    if nchunk:
        sl = slot_in.reshape(CORES, nchunk, 128).transpose(0, 2, 1)
        nr = norm_in.reshape(CORES, nchunk, 128).transpose(0, 2, 1)
        ns_inp[:, :, 0::2] = sl
        ns_inp[:, :, 1::2] = nr

    # pos gather table, quarter-remapped order, rows padded to POSW f32
    pos_pad = np.zeros((NGRP * tq, POSW), dtype=np.float32)
    pos_pad[q_n * tq + row_n, :D] = np.asarray(pos, dtype=np.float32)

    poolsel = np.zeros((CORES, TR, 128, 64 if G <= 64 else G), dtype=np.float32)
    GP = poolsel.shape[3]
    b = np.asarray(batch, dtype=np.int64)
    for c in range(CORES):
        nodes = np.arange(npc, dtype=np.int64)
        gids = b[c * npc + nodes]
        poolsel[c, nodes // 128, nodes % 128, gids] = 1.0
    cnt_g = np.bincount(b, minlength=GP).astype(np.float32)
    invcnt = (1.0 / np.maximum(cnt_g, 1.0)).astype(np.float32).reshape(GP, 1)

    meta = dict(N=N, D=D, G=G, GP=GP, npc=npc, TR=TR, TQ=TQ, T=T,
                npc_pad=npc_pad, qsize=qsize, tq=tq, plan=plan, tot=tot,
                nchunk=max(nchunk, 1))
    data = dict(idxs=idxs_inp, ns=ns_inp, pos_pad=pos_pad, poolsel=poolsel,
                invcnt=invcnt)
    return meta, data


# ------------------------------------------------------------- device build

def _build(meta, H, C, reps=1):
    import concourse.bacc as bacc
    import concourse.mybir as mybir
    from concourse.tile import TileContext
    from concourse.library_config import mlp as mlp_lib

    ABL_CONST_SEL = os.environ.get("ABL_CONST_SEL") == "1"
    ABL_NO_AG = os.environ.get("ABL_NO_AG") == "1"
    ABL_NO_GATHER = os.environ.get("ABL_NO_GATHER") == "1"

    f32 = mybir.dt.float32
    bf16 = mybir.dt.bfloat16
    i16 = mybir.dt.int16
    AF = mybir.ActivationFunctionType
    OP = mybir.AluOpType

    D = meta["D"]
    GP = meta["GP"]
    TR = meta["TR"]
    TQ = meta["TQ"]
    T = meta["T"]
    qsize = meta["qsize"]
    tq = meta["tq"]
    plan = meta["plan"]
    tot = meta["tot"]
    nchunk = meta["nchunk"]
    npc_pad = meta["npc_pad"]
    HC = C
    HH = H // 2

    nc = bacc.Bacc("TRN2", target_bir_lowering=False, debug=False,
                   num_devices=CORES)

    pos_pad_d = nc.dram_tensor("pos_pad", [NGRP * tq, POSW], f32, kind="ExternalInput")
    idxs_d = nc.dram_tensor("idxs", [128, tot // 16], i16, kind="ExternalInput")
    ns_d = nc.dram_tensor("ns", [128, nchunk * 2], f32, kind="ExternalInput")
    poolsel_d = nc.dram_tensor("poolsel", [TR, 128, GP], f32, kind="ExternalInput")
    invcnt_d = nc.dram_tensor("invcnt", [GP, 1], f32, kind="ExternalInput")
    iota_d = nc.dram_tensor("iota", [128, 128], f32, kind="ExternalInput")
    iotab_d = nc.dram_tensor("iotab", [128, 128], bf16, kind="ExternalInput")
    ident_d = nc.dram_tensor("ident", [128, 128], f32, kind="ExternalInput")
    ones_d = nc.dram_tensor("ones", [1, 128], f32, kind="ExternalInput")
    wdecl = (("W1", [D, H]), ("W2", [H, H]), ("W3", [H, H]), ("Wl1", [H, HH]),
             ("Wl2", [HH, HC]), ("b1", [1, H]), ("b2", [1, H]), ("b3", [1, H]),
             ("bl1", [1, HH]), ("bl2", [1, HC]))
    wd = {nm: nc.dram_tensor(nm, shp, f32, kind="ExternalInput")
          for nm, shp in wdecl}
    out_d = nc.dram_tensor("out", [GP, HC], f32, kind="ExternalOutput")

    hb = {}
    ha = {}
    for l in (2, 3):
        hb[l] = [nc.dram_tensor(f"hbounce{l}{q}", [qsize, H], bf16)
                 for q in range(NGRP)]
        ha[l] = [nc.dram_tensor(f"hall{l}{q}", [tq, H], bf16,
                                addr_space="Shared") for q in range(NGRP)]
    pool_b = nc.dram_tensor("pool_b", [GP, H], f32)
    pool_r = nc.dram_tensor("pool_r", [GP, H], f32, addr_space="Shared")

    with TileContext(nc) as tc:
        nc.gpsimd.load_library(mlp_lib)
        with (
            tc.tile_pool(name="const", bufs=1) as constp,
            tc.tile_pool(name="idx", bufs=1) as idxp,
            tc.tile_pool(name="xt", bufs=2) as xtp,
            tc.tile_pool(name="acc", bufs=1) as accp,
            tc.tile_pool(name="stage", bufs=1) as stagep,
            tc.tile_pool(name="msg", bufs=16) as msgp,
            tc.tile_pool(name="sel", bufs=6) as selp,
            tc.tile_pool(name="xtile", bufs=3) as xtilep,
            tc.tile_pool(name="small", bufs=4) as smallp,
            tc.tile_pool(name="psum_seg", bufs=4, space="PSUM") as psum_seg,
            tc.tile_pool(name="psum_tr", bufs=2, space="PSUM") as psum_tr,
            tc.tile_pool(name="psum_h", bufs=2, space="PSUM") as psum_h,
        ):
            iota = constp.tile([128, 128], f32)
            nc.sync.dma_start(out=iota[:], in_=iota_d[:, :])
            iotab = constp.tile([128, 128], bf16)
            nc.sync.dma_start(out=iotab[:], in_=iotab_d[:, :])
            ident = constp.tile([128, 128], f32)
            nc.sync.dma_start(out=ident[:], in_=ident_d[:, :])
            ones = constp.tile([1, 128], f32)
            nc.sync.dma_start(out=ones[:], in_=ones_d[:, :])
            Ws = {}
            for nm, shp in wdecl:
                w = constp.tile(shp, f32, tag=f"w_{nm}")
                nc.sync.dma_start(out=w[:], in_=wd[nm][:, :])
                Ws[nm] = w
            poolsel = constp.tile([128, TR, GP], f32)
            nc.sync.dma_start(out=poolsel[:],
                              in_=poolsel_d.ap().rearrange("t p g -> p t g"))
            invcnt = constp.tile([GP, 1], f32)
            nc.sync.dma_start(out=invcnt[:], in_=invcnt_d[:, :])
            idxs = idxp.tile([128, tot // 16], i16)
            nc.sync.dma_start(out=idxs[:], in_=idxs_d[:, :])
            nstile = idxp.tile([128, nchunk * 2], f32)
            nc.sync.dma_start(out=nstile[:], in_=ns_d[:, :])

            xT = None

            def edge_phase(layer, tables, elem, b_name, finish_tile):
                """Group-major sweep; finish_tile(t, acc_slice) after quarter 3."""
                W_ = H if layer > 1 else D
                acc = accp.tile([128, TR, W_], f32,
                                tag="acc" if layer > 1 else "acc1")
                chunk_j = 0
                idx_off = 0
                for (g, t, sizes) in plan:
                    ps = None
                    first = True
                    if layer > 1 and g == 0:
                        ps = psum_seg.tile([128, W_], f32, tag="seg")
                        nc.tensor.matmul(ps[:], ones[:1, :128],
                                         Ws[b_name][:1, :], start=True,
                                         stop=False)
                        first = False
                    if sizes and ps is None:
                        ps = psum_seg.tile([128, W_], f32, tag="seg")
                    tab = tables[g]
                    nsz = len(sizes)
                    for si, ni in enumerate(sizes):
                        nb = ni // 128
                        mdt = f32 if layer == 1 else bf16
                        m = msgp.tile([128, nb, elem], mdt,
                                      tag="msg" if layer == 1 else "msgb")
                        if not ABL_NO_GATHER:
                            nc.gpsimd.dma_gather(
                                m[:], tab[:, :],
                                idxs[:, idx_off // 16:(idx_off + ni) // 16],
                                ni, ni, elem)
                        idx_off += ni
                        for cc in range(nb):
                            if ABL_CONST_SEL:
                                sel = iota if layer == 1 else iotab
                            elif layer == 1:
                                sel = selp.tile([128, 128], f32, tag="sel")
                                nc.vector.tensor_scalar(
                                    out=sel[:], in0=iota[:],
                                    scalar1=nstile[:, 2 * chunk_j:2 * chunk_j + 1],
                                    scalar2=nstile[:, 2 * chunk_j + 1:2 * chunk_j + 2],
                                    op0=OP.is_equal, op1=OP.mult)
                            else:
                                sel = selp.tile([128, 128], bf16, tag="selb")
                                nc.vector.tensor_scalar(
                                    out=sel[:], in0=iotab[:],
                                    scalar1=nstile[:, 2 * chunk_j:2 * chunk_j + 1],
                                    scalar2=nstile[:, 2 * chunk_j + 1:2 * chunk_j + 2],
                                    op0=OP.is_equal, op1=OP.mult)
                            last = (si == nsz - 1 and cc == nb - 1)
                            rhs = m[:, cc, :] if layer > 1 else m[:, cc, 0:D]
                            nc.tensor.matmul(ps[:], sel[:], rhs,
                                             start=first, stop=last)
                            first = False
                            chunk_j += 1
                    # drain partial into SBUF accumulator
                    a = acc[:, t, :]
                    if ps is not None:
                        if g == 0:
                            nc.scalar.activation(a, ps[:], AF.Copy)
                        else:
                            nc.vector.tensor_add(out=a, in0=a, in1=ps[:])
                    elif g == 0:
                        nc.vector.memset(a, 0.0)
                    if g == NGRP - 1:
                        finish_tile(t, a)

            def to_xT(t, xt, xT_buf):
                tr = psum_tr.tile([128, H], f32, tag="tr")
                nc.tensor.transpose(tr[:], xt[:], ident[:])
                nc.scalar.activation(xT_buf[:, t * 128:(t + 1) * 128], tr[:],
                                     AF.Copy)

            for _rep in range(reps):
                # ================= layer 1 =================
                posq = [pos_pad_d[q * tq:(q + 1) * tq, :] for q in range(NGRP)]
                xT = xtp.tile([128, npc_pad], f32, tag="xT")
                if T > TR:
                    nc.vector.memset(xT[:, TR * 128:], 0.0)

                def finish_l1(t, a, xT_buf=xT):
                    aggT_ps = psum_tr.tile([128, 128], f32, tag="tr")
                    nc.tensor.transpose(aggT_ps[0:D, :], a, ident[:])
                    aggT = smallp.tile([D, 128], f32, tag="aggTs")
                    nc.scalar.activation(aggT[:], aggT_ps[0:D, :], AF.Copy)
                    ps2 = psum_h.tile([128, H], f32, tag="h")
                    nc.tensor.matmul(ps2[:], aggT[:, :], Ws["W1"][:, :],
                                     start=True, stop=False)
                    nc.tensor.matmul(ps2[:], ones[:1, :128], Ws["b1"][:1, :],
                                     start=False, stop=True)
                    xt = xtilep.tile([128, H], f32, tag="xt")
                    nc.scalar.activation(xt[:], ps2[:], AF.Relu)
                    to_xT(t, xt, xT_buf)

                edge_phase(1, posq, POSW, None, finish_l1)

                # ============== layers 2 and 3 ==============
                for layer, Wn, bn in ((2, "W2", "b2"), (3, "W3", "b3")):
                    stages = [stagep.tile([128, TQ, H], bf16, tag=f"st{q}",
                                           name=f"stage{q}")
                              for q in range(NGRP)]
                    for t in range(T):
                        hp = psum_h.tile([128, H], f32, tag="h")
                        nc.tensor.matmul(hp[:], xT[:, t * 128:(t + 1) * 128],
                                         Ws[Wn][:, :], start=True, stop=True)
                        nc.scalar.activation(stages[t // TQ][:, t % TQ, :],
                                             hp[:], AF.Copy)
                    for q in range(NGRP):
                        nc.sync.dma_start(
                            out=hb[layer][q].ap().rearrange(
                                "(t p) f -> p t f", p=128),
                            in_=stages[q][:])
                        if not ABL_NO_AG:
                            nc.gpsimd.collective_compute(
                                "AllGather", mybir.AluOpType.bypass,
                                replica_groups=[list(range(CORES))],
                                ins=[hb[layer][q].ap().opt()],
                                outs=[ha[layer][q].ap().opt()])

                    if layer == 2:
                        xT2 = xtp.tile([128, npc_pad], f32, tag="xT")
                        if T > TR:
                            nc.vector.memset(xT2[:, TR * 128:], 0.0)

                        def finish(t, a, xT_buf=xT2):
                            xt = xtilep.tile([128, H], f32, tag="xt")
                            nc.scalar.activation(xt[:], a, AF.Relu)
                            to_xT(t, xt, xT_buf)
                    else:
                        pp = psum_h.tile([GP, H], f32, tag="h")

                        def finish(t, a, pp=pp):
                            xt = xtilep.tile([128, H], f32, tag="xt")
                            nc.scalar.activation(xt[:], a, AF.Relu)
                            nc.tensor.matmul(pp[:], poolsel[:, t, :], xt[:],
                                             start=(t == 0), stop=(t == TR - 1))
                            if t == TR - 1:
                                psb = smallp.tile([GP, H], f32, tag="psb")
                                nc.scalar.activation(psb[:], pp[:], AF.Copy)
                                nc.sync.dma_start(out=pool_b[:, :], in_=psb[:])

                    edge_phase(layer, ha[layer], H, bn, finish)
                    if layer == 2:
                        xT = xT2

                # ================= pool + head =================
                nc.gpsimd.collective_compute(
                    "AllReduce", mybir.AluOpType.add,
                    replica_groups=[list(range(CORES))],
                    ins=[pool_b.ap().opt()], outs=[pool_r.ap().opt()])
                pooled = smallp.tile([GP, H], f32, tag="pooled")
                nc.sync.dma_start(out=pooled[:], in_=pool_r[:, :])
                gmean = smallp.tile([GP, H], f32, tag="gmean")
                nc.scalar.activation(gmean[:], pooled[:], AF.Copy,
                                     scale=invcnt[:, 0:1])
                gT_ps = psum_tr.tile([128, GP], f32, tag="tr")
                nc.tensor.transpose(gT_ps[:], gmean[:], ident[0:GP, 0:GP])
                gT = smallp.tile([H, GP], f32, tag="gTs")
                nc.scalar.activation(gT[:], gT_ps[:, 0:GP], AF.Copy)
                hh_ps = psum_h.tile([GP, HH], f32, tag="h")
                nc.tensor.matmul(hh_ps[:], gT[:, :], Ws["Wl1"][:, :],
                                 start=True, stop=False)
                nc.tensor.matmul(hh_ps[:], ones[:1, 0:GP], Ws["bl1"][:1, :],
                                 start=False, stop=True)
                hh = smallp.tile([GP, HH], f32, tag="hhs")
                nc.scalar.activation(hh[:], hh_ps[:], AF.Relu)
                hhT_ps = psum_tr.tile([HH, GP], f32, tag="tr")
                nc.tensor.transpose(hhT_ps[:], hh[:], ident[0:GP, 0:GP])
                hhT = smallp.tile([HH, GP], f32, tag="hhTs")
                nc.scalar.activation(hhT[:], hhT_ps[:], AF.Copy)
                o_ps = psum_h.tile([GP, HC], f32, tag="h")
                nc.tensor.matmul(o_ps[:], hhT[:, :], Ws["Wl2"][:, :],
                                 start=True, stop=False)
                nc.tensor.matmul(o_ps[:], ones[:1, 0:GP], Ws["bl2"][:1, :],
                                 start=False, stop=True)
                osb = smallp.tile([GP, HC], f32, tag="osb")
                nc.scalar.activation(osb[:], o_ps[:], AF.Copy)
                nc.sync.dma_start(out=out_d[:, :], in_=osb[:])

    nc.compile()
    return nc


# ----------------------------------------------------------------- entry

def kernel(pos, edge_index, batch, W1, b1, W2, b2, W3, b3, Wl1, bl1, Wl2, bl2,
           num_graphs):
    from concourse.bass_utils import run_bass_kernel_spmd

    pos = np.asarray(pos, dtype=np.float32)
    edge_index = np.asarray(edge_index)
    batch = np.asarray(batch)
    G = int(num_graphs)
    H = np.asarray(W2).shape[0]
    C = np.asarray(Wl2).shape[1]

    import sys, time as _time
    _t0 = _time.time()
    meta, data = _preprocess(pos, edge_index, batch, G)
    print(f"[kernel] preprocess done {_time.time()-_t0:.1f}s tot={meta['tot']}",
          file=sys.stderr, flush=True)
    nc = _build(meta, H, C)
    print(f"[kernel] build+compile done {_time.time()-_t0:.1f}s",
          file=sys.stderr, flush=True)

    iota = np.tile(np.arange(128, dtype=np.float32)[None, :], (128, 1))
    base = {
        "pos_pad": data["pos_pad"],
        "invcnt": data["invcnt"],
        "iota": iota, "iotab": iota.astype(ml_dtypes.bfloat16),
        "ident": np.eye(128, dtype=np.float32),
        "ones": np.ones((1, 128), np.float32),
        "W1": np.asarray(W1, np.float32), "W2": np.asarray(W2, np.float32),
        "W3": np.asarray(W3, np.float32), "Wl1": np.asarray(Wl1, np.float32),
        "Wl2": np.asarray(Wl2, np.float32),
        "b1": np.asarray(b1, np.float32).reshape(1, -1),
        "b2": np.asarray(b2, np.float32).reshape(1, -1),
        "b3": np.asarray(b3, np.float32).reshape(1, -1),
        "bl1": np.asarray(bl1, np.float32).reshape(1, -1),
        "bl2": np.asarray(bl2, np.float32).reshape(1, -1),
    }
    in_maps = []
    for c in range(CORES):
        m = dict(base)
        m["idxs"] = data["idxs"][c]
        m["ns"] = data["ns"][c]
        m["poolsel"] = data["poolsel"][c]
        in_maps.append(m)

    print("[kernel] executing", file=sys.stderr, flush=True)
    res = run_bass_kernel_spmd(nc, in_maps, core_ids=list(range(CORES)))
    print(f"[kernel] exec done {_time.time()-_t0:.1f}s", file=sys.stderr,
          flush=True)
    global LAST_EXEC_NS, LAST_RESULT
    LAST_EXEC_NS = res.exec_time_ns
    LAST_RESULT = res
    out = res.results[0]["out"][:G].astype(np.float32)
    return out


LAST_EXEC_NS = None
LAST_RESULT = None



# revision 11
# speedup vs baseline: 20.0736x; 20.0736x over previous
"""GCN (3-layer) + mean-pool + MLP head on 8 Trainium2 NeuronCores.

Strategy (data-parallel over dst nodes):
- Nodes are partitioned into 8 contiguous ranges (one per core). Each core owns
  all edges whose dst falls in its range (plus self-loops), so the scatter side
  of message passing is core-local.
- Per layer: each core computes h = x @ W for its own nodes (bf16), the shards
  are AllGather'd into full node-major tables in DRAM, and per-edge messages
  h[src] are fetched with dma_gather (SWDGE indexed DMA, 256B rows). Gather
  calls rotate across 4 SWDGE queues so descriptor generation for call k+1
  overlaps the SDMA drain of call k (one 1024-descriptor ring per queue).
- Per-core real edge counts are passed to each gather via num_idxs_reg
  (padding slots carry index -1 and are skipped by the Q7 descriptor loop).
- Segment-sum by dst runs on the tensor engine: for each 128-edge chunk a
  host-built selection matrix Sel[e, slot] = norm[e] * (slot[e] == s) (bf16,
  streamed from DRAM over the idle scalar-engine HWDGE path) is matmul'd
  (Sel^T @ msgs) into a PSUM tile per 128-node dst tile; per-quarter partials
  are drained into an SBUF accumulator.
- The node table is split into FOUR tile-aligned quarters, each AllGather'd
  separately, and edges are swept quarter-major: quarter q's edge processing
  overlaps the AllGather of quarter q+1, and the h-matmul + first AllGather of
  the next layer overlaps the tail of the previous edge sweep.
- Layer 1 aggregates A@pos first (pos quantized to bf16 in a 256B-row table),
  then pushes the 3-wide aggregate through W1. The same sel table works for
  all three layers.
- Graph mean-pool: per-core partial sums via matmul against a host-built
  node->graph selection, AllReduce, then the tiny MLP head runs redundantly
  on every core.
- The SAME NEFF runs on all 8 cores (SPMD): per-(tile, quarter) edge counts
  are padded to the max across cores so the instruction schedule is identical
  everywhere; padded slots have all-zero sel columns so they contribute
  nothing.
- int16 gather indices only reach 32767, so each quarter table keeps its own
  index base (quarter size * 8 cores <= 32768 rows).
"""

import os
import numpy as np
import ml_dtypes

CORES = 8
NGRP = 4      # table quarters
MAXNI = 1024  # max indices per dma_gather call (descriptor-ring limit)
NQ = int(os.environ.get("K_NQ", "4"))   # SWDGE queues, round-robin
FULLCNT = os.environ.get("K_FULLCNT") == "1"  # disable num_idxs_reg trick
NOREG = os.environ.get("K_NOREG") == "1"      # pass ni as plain int
SELSYNC = os.environ.get("K_SELSYNC") == "1"  # sel stream on sync engine


# ----------------------------------------------------------------- host prep

def _plan_calls(m):
    """Split m (multiple of 128) indices into balanced calls of <= MAXNI."""
    if m == 0:
        return []
    q = m // 128
    ncalls = (m + MAXNI - 1) // MAXNI
    sizes = []
    base, rem = divmod(q, ncalls)
    for i in range(ncalls):
        sizes.append((base + (1 if i < rem else 0)) * 128)
    return [s for s in sizes if s > 0]


def _preprocess(pos, edge_index, batch, num_graphs):
    N, D = pos.shape
    G = int(num_graphs)
    assert N % CORES == 0, N
    npc = N // CORES                       # real nodes per core
    TR = (npc + 127) // 128                # real dst tiles per core
    TQ = (TR + NGRP - 1) // NGRP           # tiles per quarter
    T = TQ * NGRP                          # padded tile count
    npc_pad = T * 128
    qsize = TQ * 128                       # rows per core per quarter
    tq = qsize * CORES                     # rows per quarter table
    assert tq <= 32768

    src = np.concatenate([edge_index[0], np.arange(N, dtype=np.int64)])
    dst = np.concatenate([edge_index[1], np.arange(N, dtype=np.int64)])
    deg = np.bincount(dst, minlength=N).astype(np.float32)
    dinv = (1.0 / np.sqrt(np.maximum(deg, 1e-12))).astype(np.float32)
    norm = dinv[src] * dinv[dst]

    core_of = dst // npc
    loc = dst - core_of * npc
    tile_of = loc // 128
    slot_of = loc - tile_of * 128

    # remapped quarter-table row of each node
    c_n = np.arange(N, dtype=np.int64) // npc
    i_n = np.arange(N, dtype=np.int64) - c_n * npc
    q_n = i_n // qsize
    row_n = c_n * qsize + (i_n - q_n * qsize)   # row within quarter table
    grp = q_n[src]
    idx16 = row_n[src]

    # bucket edges by (group, tile, core) -- group-major device sweep order
    order = np.lexsort((idx16, slot_of, core_of, tile_of, grp))
    core_s = core_of[order]
    tile_s = tile_of[order]
    grp_s = grp[order]
    idx_s = idx16[order]
    slot_s = slot_of[order]
    norm_s = norm[order]

    key = (grp_s * TR + tile_s) * CORES + core_s
    nk = NGRP * TR * CORES
    cnt = np.bincount(key, minlength=nk).reshape(NGRP, TR, CORES)
    M = cnt.max(axis=2)                    # [NGRP, TR]
    M = ((M + 127) // 128) * 128

    # call plan shared by all cores / layers, group-major
    plan = []                              # (g, t, [ni...])
    for g in range(NGRP):
        for t in range(TR):
            plan.append((g, t, _plan_calls(int(M[g, t]))))
    tot = int(M.sum())
    nchunk = tot // 128

    starts = np.zeros(nk + 1, dtype=np.int64)
    np.cumsum(np.bincount(key, minlength=nk), out=starts[1:])
    idx_in = np.full((CORES, tot), -1, dtype=np.int64)
    slot_in = np.zeros((CORES, tot), dtype=np.int64)
    norm_in = np.zeros((CORES, tot), dtype=np.float32)
    real_cnt = np.zeros((CORES, NGRP, TR), dtype=np.int64)
    for c in range(CORES):
        off = 0
        for g in range(NGRP):
            for t in range(TR):
                k = (g * TR + t) * CORES + c
                s, e = starts[k], starts[k + 1]
                m = int(M[g, t])
                idx_in[c, off:off + (e - s)] = idx_s[s:e]
                slot_in[c, off:off + (e - s)] = slot_s[s:e]
                norm_in[c, off:off + (e - s)] = norm_s[s:e]
                real_cnt[c, g, t] = e - s
                off += m

    # per-call true counts (>=16 so every DMA engine gets work for the
    # completion semaphore); count-covered pad slots use valid index 0.
    counts = []           # [ncalls] per core below
    call_meta = []        # (g, t, ni, off)
    off = 0
    for (g, t, sizes) in plan:
        boff = 0
        for ni in sizes:
            call_meta.append((g, t, ni, off))
            off += ni
            boff += ni
    ncalls = len(call_meta)
    counts_inp = np.zeros((CORES, 1, ncalls), dtype=np.int32)
    for c in range(CORES):
        bucket_used = {}
        for j, (g, t, ni, off) in enumerate(call_meta):
            used = bucket_used.get((g, t), 0)
            real = int(min(max(real_cnt[c, g, t] - used, 0), ni))
            bucket_used[(g, t)] = used + ni
            cntv = max(real, min(16, ni))
            if cntv > real:
                idx_in[c, off + real:off + cntv] = 0
            counts_inp[c, 0, j] = cntv
    if FULLCNT:
        idx_in[idx_in < 0] = 0
        for j, (g, t, ni, off) in enumerate(call_meta):
            counts_inp[:, 0, j] = ni

    # wrap indices per call: position i of a call reads tile16[i%16, i//16]
    idx_wrapped = np.zeros((CORES, 16, tot // 16), dtype=np.int16)
    col = 0
    off = 0
    for (g, t, sizes) in plan:
        for ni in sizes:
            seg = idx_in[:, off:off + ni]
            w = seg.reshape(CORES, ni // 16, 16).transpose(0, 2, 1)
            idx_wrapped[:, :, col:col + ni // 16] = w.astype(np.int16)
            col += ni // 16
            off += ni
    idxs_inp = np.tile(idx_wrapped, (1, 8, 1))

    # host-built selection matrices: sel[e, slot] = norm[e] one-hot, stored
    # pre-wrapped [128, nchunk*128] bf16 so each partition's stream per call
    # is one contiguous descriptor.
    ee = np.arange(tot, dtype=np.int64)
    bidx = ee // 128
    pidx = ee % 128
    sel = np.zeros((CORES, 128, nchunk, 128), dtype=ml_dtypes.bfloat16)
    for c in range(CORES):
        sel[c, pidx, bidx, slot_in[c]] = norm_in[c].astype(ml_dtypes.bfloat16)
    sel_inp = sel.reshape(CORES, 128, nchunk * 128)

    # pos gather table, quarter-remapped order, bf16 256B rows
    pos_pad = np.zeros((NGRP * tq, 128), dtype=ml_dtypes.bfloat16)
    pos_pad[q_n * tq + row_n, :D] = np.asarray(pos, dtype=np.float32)

    poolsel = np.zeros((CORES, TR, 128, 64 if G <= 64 else G), dtype=np.float32)
    GP = poolsel.shape[3]
    b = np.asarray(batch, dtype=np.int64)
    for c in range(CORES):
        nodes = np.arange(npc, dtype=np.int64)
        gids = b[c * npc + nodes]
        poolsel[c, nodes // 128, nodes % 128, gids] = 1.0
    cnt_g = np.bincount(b, minlength=GP).astype(np.float32)
    invcnt = (1.0 / np.maximum(cnt_g, 1.0)).astype(np.float32).reshape(GP, 1)

    meta = dict(N=N, D=D, G=G, GP=GP, npc=npc, TR=TR, TQ=TQ, T=T,
                npc_pad=npc_pad, qsize=qsize, tq=tq, plan=plan, tot=tot,
                nchunk=max(nchunk, 1), ncalls=ncalls)
    data = dict(idxs=idxs_inp, sel=sel_inp, counts=counts_inp,
                pos_pad=pos_pad, poolsel=poolsel, invcnt=invcnt)
    return meta, data


# ------------------------------------------------------------- device build

def _build(meta, H, C, reps=1):
    import concourse.bacc as bacc
    import concourse.mybir as mybir
    from concourse.tile import TileContext
    from concourse.library_config import mlp as mlp_lib

    f32 = mybir.dt.float32
    bf16 = mybir.dt.bfloat16
    i16 = mybir.dt.int16
    i32 = mybir.dt.int32
    AF = mybir.ActivationFunctionType

    D = meta["D"]
    GP = meta["GP"]
    TR = meta["TR"]
    TQ = meta["TQ"]
    T = meta["T"]
    qsize = meta["qsize"]
    tq = meta["tq"]
    plan = meta["plan"]
    tot = meta["tot"]
    nchunk = meta["nchunk"]
    ncalls = meta["ncalls"]
    npc_pad = meta["npc_pad"]
    HC = C
    HH = H // 2

    nc = bacc.Bacc("TRN2", target_bir_lowering=False, debug=False,
                   num_devices=CORES, num_swdge_queues=NQ)

    pos_pad_d = nc.dram_tensor("pos_pad", [NGRP * tq, 128], bf16, kind="ExternalInput")
    idxs_d = nc.dram_tensor("idxs", [128, tot // 16], i16, kind="ExternalInput")
    sel_d = nc.dram_tensor("sel", [128, nchunk * 128], bf16, kind="ExternalInput")
    counts_d = nc.dram_tensor("counts", [1, ncalls], i32, kind="ExternalInput")
    poolsel_d = nc.dram_tensor("poolsel", [TR, 128, GP], f32, kind="ExternalInput")
    invcnt_d = nc.dram_tensor("invcnt", [GP, 1], f32, kind="ExternalInput")
    ident_d = nc.dram_tensor("ident", [128, 128], f32, kind="ExternalInput")
    ones_d = nc.dram_tensor("ones", [1, 128], f32, kind="ExternalInput")
    wdecl = (("W1", [D, H]), ("W2", [H, H]), ("W3", [H, H]), ("Wl1", [H, HH]),
             ("Wl2", [HH, HC]), ("b1", [1, H]), ("b2", [1, H]), ("b3", [1, H]),
             ("bl1", [1, HH]), ("bl2", [1, HC]))
    wd = {nm: nc.dram_tensor(nm, shp, f32, kind="ExternalInput")
          for nm, shp in wdecl}
    out_d = nc.dram_tensor("out", [GP, HC], f32, kind="ExternalOutput")

    hb = {}
    ha = {}
    for l in (2, 3):
        hb[l] = [nc.dram_tensor(f"hbounce{l}{q}", [qsize, H], bf16)
                 for q in range(NGRP)]
        ha[l] = [nc.dram_tensor(f"hall{l}{q}", [tq, H], bf16,
                                addr_space="Shared") for q in range(NGRP)]
    pool_b = nc.dram_tensor("pool_b", [GP, H], f32)
    pool_r = nc.dram_tensor("pool_r", [GP, H], f32, addr_space="Shared")

    with TileContext(nc) as tc:
        nc.gpsimd.load_library(mlp_lib)
        with (
            tc.tile_pool(name="const", bufs=1) as constp,
            tc.tile_pool(name="idx", bufs=1) as idxp,
            tc.tile_pool(name="xt", bufs=2) as xtp,
            tc.tile_pool(name="acc", bufs=1) as accp,
            tc.tile_pool(name="stage", bufs=1) as stagep,
            tc.tile_pool(name="msg", bufs=16) as msgp,
            tc.tile_pool(name="sel", bufs=6) as selp,
            tc.tile_pool(name="xtile", bufs=3) as xtilep,
            tc.tile_pool(name="small", bufs=4) as smallp,
            tc.tile_pool(name="psum_seg", bufs=4, space="PSUM") as psum_seg,
            tc.tile_pool(name="psum_tr", bufs=2, space="PSUM") as psum_tr,
            tc.tile_pool(name="psum_h", bufs=2, space="PSUM") as psum_h,
        ):
            ident = constp.tile([128, 128], f32)
            nc.sync.dma_start(out=ident[:], in_=ident_d[:, :])
            ones = constp.tile([1, 128], f32)
            nc.sync.dma_start(out=ones[:], in_=ones_d[:, :])
            Ws = {}
            for nm, shp in wdecl:
                w = constp.tile(shp, f32, tag=f"w_{nm}")
                nc.sync.dma_start(out=w[:], in_=wd[nm][:, :])
                Ws[nm] = w
            poolsel = constp.tile([128, TR, GP], f32)
            nc.sync.dma_start(out=poolsel[:],
                              in_=poolsel_d.ap().rearrange("t p g -> p t g"))
            invcnt = constp.tile([GP, 1], f32)
            nc.sync.dma_start(out=invcnt[:], in_=invcnt_d[:, :])
            idxs = idxp.tile([128, tot // 16], i16)
            nc.sync.dma_start(out=idxs[:], in_=idxs_d[:, :])
            counts = idxp.tile([1, ncalls], i32)
            nc.sync.dma_start(out=counts[:], in_=counts_d[:, :])

            # warm all msg buffers: gathers skip slots past the per-core edge
            # count, and stale-SBUF NaN bit patterns would poison 0*garbage.
            # The warmed tiles are consumed by the first 16 gather calls so
            # the memsets have live uses (DCE would drop unread tiles).
            MB = MAXNI // 128
            warm = []
            for _ in range(16):
                mm = msgp.tile([128, MB, 128], bf16, tag="msgb")
                nc.vector.memset(mm[:], 0.0)
                warm.append(mm)

            xT = None

            def edge_phase(layer, tables, b_name, finish_tile):
                """Group-major sweep; finish_tile(t, acc_slice) after quarter 3."""
                W_ = H if layer > 1 else D
                acc = accp.tile([128, TR, W_], f32,
                                tag="acc" if layer > 1 else "acc1")
                idx_off = 0
                call_j = 0
                for (g, t, sizes) in plan:
                    ps = None
                    first = True
                    if layer > 1 and g == 0:
                        ps = psum_seg.tile([128, W_], f32, tag="seg")
                        nc.tensor.matmul(ps[:], ones[:1, :128],
                                         Ws[b_name][:1, :], start=True,
                                         stop=False)
                        first = False
                    if sizes and ps is None:
                        ps = psum_seg.tile([128, W_], f32, tag="seg")
                    tab = tables[g]
                    nsz = len(sizes)
                    for si, ni in enumerate(sizes):
                        nb = ni // 128
                        boff = idx_off // 128
                        m = warm.pop(0) if warm else msgp.tile(
                            [128, MAXNI // 128, 128], bf16, tag="msgb")
                        if NOREG:
                            cnt_reg = ni
                        else:
                            cnt_reg = nc.gpsimd.value_load(
                                counts[0:1, call_j:call_j + 1],
                                min_val=1, max_val=ni)
                        nc.gpsimd.dma_gather(
                            m[:, 0:nb, :], tab[:, :],
                            idxs[:, idx_off // 16:(idx_off + ni) // 16],
                            ni, cnt_reg, 128, queue_num=call_j % NQ)
                        st = selp.tile([128, nb * 128], bf16, tag="selb")
                        seleng = nc.sync if SELSYNC else nc.scalar
                        seleng.dma_start(
                            out=st[:],
                            in_=sel_d[:, boff * 128:(boff + nb) * 128])
                        idx_off += ni
                        call_j += 1
                        for cc in range(nb):
                            last = (si == nsz - 1 and cc == nb - 1)
                            rhs = m[:, cc, :] if layer > 1 else m[:, cc, 0:D]
                            nc.tensor.matmul(ps[:],
                                             st[:, cc * 128:(cc + 1) * 128],
                                             rhs, start=first, stop=last)
                            first = False
                    # drain partial into SBUF accumulator
                    a = acc[:, t, :]
                    if ps is not None:
                        if g == 0:
                            nc.scalar.activation(a, ps[:], AF.Copy)
                        else:
                            nc.vector.tensor_add(out=a, in0=a, in1=ps[:])
                    elif g == 0:
                        nc.vector.memset(a, 0.0)
                    if g == NGRP - 1:
                        finish_tile(t, a)

            def to_xT(t, xt, xT_buf):
                tr = psum_tr.tile([128, H], f32, tag="tr")
                nc.tensor.transpose(tr[:], xt[:], ident[:])
                nc.scalar.activation(xT_buf[:, t * 128:(t + 1) * 128], tr[:],
                                     AF.Copy)

            for _rep in range(reps):
                # ================= layer 1 =================
                posq = [pos_pad_d[q * tq:(q + 1) * tq, :] for q in range(NGRP)]
                xT = xtp.tile([128, npc_pad], f32, tag="xT")
                if T > TR:
                    nc.vector.memset(xT[:, TR * 128:], 0.0)

                def finish_l1(t, a, xT_buf=xT):
                    aggT_ps = psum_tr.tile([128, 128], f32, tag="tr")
                    nc.tensor.transpose(aggT_ps[0:D, :], a, ident[:])
                    aggT = smallp.tile([D, 128], f32, tag="aggTs")
                    nc.scalar.activation(aggT[:], aggT_ps[0:D, :], AF.Copy)
                    ps2 = psum_h.tile([128, H], f32, tag="h")
                    nc.tensor.matmul(ps2[:], aggT[:, :], Ws["W1"][:, :],
                                     start=True, stop=False)
                    nc.tensor.matmul(ps2[:], ones[:1, :128], Ws["b1"][:1, :],
                                     start=False, stop=True)
                    xt = xtilep.tile([128, H], f32, tag="xt")
                    nc.scalar.activation(xt[:], ps2[:], AF.Relu)
                    to_xT(t, xt, xT_buf)

                edge_phase(1, posq, None, finish_l1)

                # ============== layers 2 and 3 ==============
                for layer, Wn, bn in ((2, "W2", "b2"), (3, "W3", "b3")):
                    stages = [stagep.tile([128, TQ, H], bf16, tag=f"st{q}",
                                           name=f"stage{q}")
                              for q in range(NGRP)]
                    for t in range(T):
                        hp = psum_h.tile([128, H], f32, tag="h")
                        nc.tensor.matmul(hp[:], xT[:, t * 128:(t + 1) * 128],
                                         Ws[Wn][:, :], start=True, stop=True)
                        nc.scalar.activation(stages[t // TQ][:, t % TQ, :],
                                             hp[:], AF.Copy)
                    for q in range(NGRP):
                        nc.sync.dma_start(
                            out=hb[layer][q].ap().rearrange(
                                "(t p) f -> p t f", p=128),
                            in_=stages[q][:])
                        nc.gpsimd.collective_compute(
                            "AllGather", mybir.AluOpType.bypass,
                            replica_groups=[list(range(CORES))],
                            ins=[hb[layer][q].ap().opt()],
                            outs=[ha[layer][q].ap().opt()])

                    if layer == 2:
                        xT2 = xtp.tile([128, npc_pad], f32, tag="xT")
                        if T > TR:
                            nc.vector.memset(xT2[:, TR * 128:], 0.0)

                        def finish(t, a, xT_buf=xT2):
                            xt = xtilep.tile([128, H], f32, tag="xt")
                            nc.scalar.activation(xt[:], a, AF.Relu)
                            to_xT(t, xt, xT_buf)
                    else:
                        pp = psum_h.tile([GP, H], f32, tag="h")

                        def finish(t, a, pp=pp):
                            xt = xtilep.tile([128, H], f32, tag="xt")
                            nc.scalar.activation(xt[:], a, AF.Relu)
                            nc.tensor.matmul(pp[:], poolsel[:, t, :], xt[:],
                                             start=(t == 0), stop=(t == TR - 1))
                            if t == TR - 1:
                                psb = smallp.tile([GP, H], f32, tag="psb")
                                nc.scalar.activation(psb[:], pp[:], AF.Copy)
                                nc.sync.dma_start(out=pool_b[:, :], in_=psb[:])

                    edge_phase(layer, ha[layer], bn, finish)
                    if layer == 2:
                        xT = xT2

                # ================= pool + head =================
                nc.gpsimd.collective_compute(
                    "AllReduce", mybir.AluOpType.add,
                    replica_groups=[list(range(CORES))],
                    ins=[pool_b.ap().opt()], outs=[pool_r.ap().opt()])
                pooled = smallp.tile([GP, H], f32, tag="pooled")
                nc.sync.dma_start(out=pooled[:], in_=pool_r[:, :])
                gmean = smallp.tile([GP, H], f32, tag="gmean")
                nc.scalar.activation(gmean[:], pooled[:], AF.Copy,
                                     scale=invcnt[:, 0:1])
                gT_ps = psum_tr.tile([128, GP], f32, tag="tr")
                nc.tensor.transpose(gT_ps[:], gmean[:], ident[0:GP, 0:GP])
                gT = smallp.tile([H, GP], f32, tag="gTs")
                nc.scalar.activation(gT[:], gT_ps[:, 0:GP], AF.Copy)
                hh_ps = psum_h.tile([GP, HH], f32, tag="h")
                nc.tensor.matmul(hh_ps[:], gT[:, :], Ws["Wl1"][:, :],
                                 start=True, stop=False)
                nc.tensor.matmul(hh_ps[:], ones[:1, 0:GP], Ws["bl1"][:1, :],
                                 start=False, stop=True)
                hh = smallp.tile([GP, HH], f32, tag="hhs")
                nc.scalar.activation(hh[:], hh_ps[:], AF.Relu)
                hhT_ps = psum_tr.tile([HH, GP], f32, tag="tr")
                nc.tensor.transpose(hhT_ps[:], hh[:], ident[0:GP, 0:GP])
                hhT = smallp.tile([HH, GP], f32, tag="hhTs")
                nc.scalar.activation(hhT[:], hhT_ps[:], AF.Copy)
                o_ps = psum_h.tile([GP, HC], f32, tag="h")
                nc.tensor.matmul(o_ps[:], hhT[:, :], Ws["Wl2"][:, :],
                                 start=True, stop=False)
                nc.tensor.matmul(o_ps[:], ones[:1, 0:GP], Ws["bl2"][:1, :],
                                 start=False, stop=True)
                osb = smallp.tile([GP, HC], f32, tag="osb")
                nc.scalar.activation(osb[:], o_ps[:], AF.Copy)
                nc.sync.dma_start(out=out_d[:, :], in_=osb[:])

    nc.compile()
    return nc


# ----------------------------------------------------------------- entry

def kernel(pos, edge_index, batch, W1, b1, W2, b2, W3, b3, Wl1, bl1, Wl2, bl2,
           num_graphs):
    from concourse.bass_utils import run_bass_kernel_spmd

    pos = np.asarray(pos, dtype=np.float32)
    edge_index = np.asarray(edge_index)
    batch = np.asarray(batch)
    G = int(num_graphs)
    H = np.asarray(W2).shape[0]
    C = np.asarray(Wl2).shape[1]

    import sys, time as _time
    _t0 = _time.time()
    meta, data = _preprocess(pos, edge_index, batch, G)
    print(f"[kernel] preprocess done {_time.time()-_t0:.1f}s tot={meta['tot']}",
          file=sys.stderr, flush=True)
    nc = _build(meta, H, C)
    print(f"[kernel] build+compile done {_time.time()-_t0:.1f}s",
          file=sys.stderr, flush=True)

    base = {
        "pos_pad": data["pos_pad"],
        "invcnt": data["invcnt"],
        "ident": np.eye(128, dtype=np.float32),
        "ones": np.ones((1, 128), np.float32),
        "W1": np.asarray(W1, np.float32), "W2": np.asarray(W2, np.float32),
        "W3": np.asarray(W3, np.float32), "Wl1": np.asarray(Wl1, np.float32),
        "Wl2": np.asarray(Wl2, np.float32),
        "b1": np.asarray(b1, np.float32).reshape(1, -1),
        "b2": np.asarray(b2, np.float32).reshape(1, -1),
        "b3": np.asarray(b3, np.float32).reshape(1, -1),
        "bl1": np.asarray(bl1, np.float32).reshape(1, -1),
        "bl2": np.asarray(bl2, np.float32).reshape(1, -1),
    }
    in_maps = []
    for c in range(CORES):
        m = dict(base)
        m["idxs"] = data["idxs"][c]
        m["sel"] = data["sel"][c]
        m["counts"] = data["counts"][c]
        m["poolsel"] = data["poolsel"][c]
        in_maps.append(m)

    print("[kernel] executing", file=sys.stderr, flush=True)
    res = run_bass_kernel_spmd(nc, in_maps, core_ids=list(range(CORES)))
    print(f"[kernel] exec done {_time.time()-_t0:.1f}s", file=sys.stderr,
          flush=True)
    global LAST_EXEC_NS, LAST_RESULT
    LAST_EXEC_NS = res.exec_time_ns
    LAST_RESULT = res
    out = res.results[0]["out"][:G].astype(np.float32)
    return out


LAST_EXEC_NS = None
LAST_RESULT = None
